# revision 2
# baseline (speedup 1.0000x reference)
"""GroupTopk Trainium2 kernel, v2.

x: [16, 512, 64, 64] f32. Per pixel, per group of 32 channels: top-4 values
(descending), grouped 1x1 conv [4 -> 32] with per-group weight w[g, o, k],
residual add. out = x + enhanced.

Strategy (8 cores, data-parallel over N, 2 images/core), per 512-pixel batch:
 - DMA x channel-major (contiguous 2KB lines) into an f32r-typed tile.
 - PE transposes (f32r, 1.5 cyc/row) into PSUM pixel-major.
 - Act evacuates PSUM -> SBUF as fp16 in "val-major" layout: plane v (channel
   within group) outer, unit u = (pixel-block, group) inner, so every sort op
   has stride-1 innermost access -> DVE 2x packed mode.
 - Bitonic top-4 merge network on DVE in fp16 (23 ops, all 2x-eligible).
 - PE transposes top-4 planes back (fp16, 1 cyc/row), fp16 conv matmul with
   block-diagonal weight + f32r identity residual-accumulate into PSUM.
 - Act evacuates PSUM -> SBUF f32; DMA out channel-major.
Queues: DVE = sort only; Act = evacuations; PE = transpose/matmul; SP = DMA.
"""

import numpy as np
from contextlib import ExitStack

import concourse.bacc as bacc
import concourse.bass as bass
import concourse.mybir as mybir
import concourse.tile as tile
from concourse.bass_utils import run_bass_kernel_spmd

F32 = mybir.dt.float32
F32R = mybir.dt.float32r
F16 = mybir.dt.float16

N, C, H, W = 16, 512, 64, 64
HW = H * W            # 4096
G, GS, K = 16, 32, 4  # groups, group size, topk
NCORES = 8
IMGS = N // NCORES    # images per core
PB = 4                # 128-pixel blocks per batch
BPX = PB * 128        # 512 pixels per batch
NBATCH = IMGS * HW // BPX

MAX = mybir.AluOpType.max
MIN = mybir.AluOpType.min


def _v(t, off, dims):
    """Strided view of a tile: keep partition dim, set free dims."""
    b = t[:]
    return bass.AP(
        tensor=b.tensor,
        offset=b.offset + off,
        ap=[list(b.ap[0])] + [list(d) for d in dims],
    )


def _build_nc():
    nc = bacc.Bacc("TRN2", target_bir_lowering=False, debug=False)
    x_d = nc.dram_tensor("x", [IMGS, C, HW], F32R, kind="ExternalInput").ap()
    w_d = nc.dram_tensor("wbd2", [64, C], F16, kind="ExternalInput").ap()
    ir_d = nc.dram_tensor("identr", [128, 128], F32R, kind="ExternalInput").ap()
    ih_d = nc.dram_tensor("identh", [128, 128], F16, kind="ExternalInput").ap()
    out_d = nc.dram_tensor("out", [IMGS, C, HW], F32, kind="ExternalOutput").ap()

    with tile.TileContext(nc) as tc:
        with ExitStack() as ctx:
            _emit(ctx, tc, nc, out_d, x_d, w_d, ir_d, ih_d)
    nc.compile()
    return nc


def _emit(ctx, tc, nc, out_d, x_d, w_d, ir_d, ih_d):
    consts = ctx.enter_context(tc.tile_pool(name="consts", bufs=1))
    xpool = ctx.enter_context(tc.tile_pool(name="xt", bufs=6))
    vpool = ctx.enter_context(tc.tile_pool(name="xv", bufs=3))
    wpool = ctx.enter_context(tc.tile_pool(name="work", bufs=3))
    mpool = ctx.enter_context(tc.tile_pool(name="mt", bufs=2))
    opool = ctx.enter_context(tc.tile_pool(name="osb", bufs=3))
    psx_pool = ctx.enter_context(tc.tile_pool(name="psx", bufs=1, space="PSUM"))
    pst_pool = ctx.enter_context(tc.tile_pool(name="pst", bufs=1, space="PSUM"))
    psc_pool = ctx.enter_context(tc.tile_pool(name="psc", bufs=3, space="PSUM"))

    # identr is needed immediately (warmup + first transposes); the other
    # consts are loaded after the first x chunks so they don't hold up the
    # serial HWDGE at kernel start
    identr = consts.tile([128, 128], F32R)
    nc.sync.dma_start(identr[:], ir_d[:])
    wbd2 = consts.tile([64, C], F16)
    identh = consts.tile([128, 128], F16)

    state = {}

    # work items: (img, px0, npx); first/last are half-size to shorten
    # pipeline fill and drain
    items = []
    items += [(0, 0, 256), (0, 256, 256)]
    items += [(0, 512 + k * 512, 512) for k in range(7)]
    items += [(1, k * 512, 512) for k in range(7)]
    items += [(1, 3584, 256), (1, 3840, 256)]
    NIT = len(items)

    def load(i, chunked=False):
        img, px0, npx = items[i]
        # xt[cc, cb*npx + px] = x[img, cb*128+cc, px0+px]
        xt = xpool.tile([128, PB * 512], F32R)
        base = x_d[:].offset + img * C * HW + px0
        if chunked:
            for cb in range(4):
                dram = bass.AP(tensor=x_d.tensor, offset=base + cb * 128 * HW,
                               ap=[[HW, 128], [1, npx]])
                nc.sync.dma_start(_v(xt, cb * npx, [[1, npx]]), dram)
        else:
            dram = bass.AP(tensor=x_d.tensor, offset=base,
                           ap=[[HW, 128], [128 * HW, 4], [1, npx]])
            nc.sync.dma_start(_v(xt, 0, [[npx, 4], [1, npx]]), dram)
        state[("xt", i)] = xt

    def tin(i):
        img, px0, npx = items[i]
        npb = npx // 128
        U = npx // 8
        xt = state[("xt", i)]
        # PE transpose -> psX[px_part, pb*512 + cb*128 + cc] (pixel-major, f32r)
        psX = psx_pool.tile([128, PB * 512], F32R)
        if i == 0:
            # dummy transposes with no DMA dependency: ramp the PE p-state
            # during the first input DMA (a cold PE runs 3.7x slower)
            for _ in range(4):
                nc.tensor.transpose(_v(psX, 0, [[1, 128]]), identr[:], identr[:])
        for cb in range(4):
            for pb in range(npb):
                src = _v(xt, cb * npx + pb * 128, [[1, 128]])   # [128cc, 128px]
                dst = _v(psX, pb * 512 + cb * 128, [[1, 128]])  # [128px, 128cc]
                nc.tensor.transpose(dst, src, identr[:])
        # Act evac: fp16 val-major scatter.  Xv[p, v*U + u], u = pb*16+cb*4+g
        Xv = vpool.tile([128, 2048], F16)
        src = _v(psX, 0, [[512, npb], [128, 4], [32, 4], [1, 32]])  # pb, cb, g, v
        dst = _v(Xv, 0, [[16, npb], [4, 4], [1, 4], [U, 32]])
        nc.scalar.copy(dst, src)
        state[("xv", i)] = Xv

    def sortb(i):
        img, px0, npx = items[i]
        npb = npx // 128
        U = npx // 8  # units per partition row
        Xv = state[("xv", i)]

        def tt(out, a, b, op):
            nc.vector.tensor_tensor(out, a, b, op)

        # ---- A: pairs -> PHL planes [PH(16) | PL(16)]
        PHL = wpool.tile([128, 2048], F16)
        Xe = _v(Xv, 0, [[2 * U, 16], [1, U]])
        Xo = _v(Xv, U, [[2 * U, 16], [1, U]])
        tt(_v(PHL, 0, [[U, 16], [1, U]]), Xe, Xo, MAX)
        tt(_v(PHL, 16 * U, [[U, 16], [1, U]]), Xe, Xo, MIN)

        # ---- B: sorted-4 lists, fused: one max op writes Y0 (rank0) + m2,
        # one min op writes m1 + Y3 (rank3); YY layout:
        # rank0 @0, rank1 @8U, rank2 @16U, rank3 @24U, m1 @32U, m2 @40U
        YY = wpool.tile([128, 3072], F16)
        He_and_Le = _v(PHL, 0, [[16 * U, 2], [2 * U, 8], [1, U]])
        Ho_and_Lo = _v(PHL, U, [[16 * U, 2], [2 * U, 8], [1, U]])
        tt(_v(YY, 0, [[40 * U, 2], [U, 8], [1, U]]),
           He_and_Le, Ho_and_Lo, MAX)                            # Y0, m2
        tt(_v(YY, 32 * U, [[-8 * U, 2], [U, 8], [1, U]]),
           He_and_Le, Ho_and_Lo, MIN)                            # m1, Y3
        m1 = _v(YY, 32 * U, [[U, 8], [1, U]])
        m2 = _v(YY, 40 * U, [[U, 8], [1, U]])
        tt(_v(YY, 8 * U, [[U, 8], [1, U]]), m1, m2, MAX)         # Y1
        tt(_v(YY, 16 * U, [[U, 8], [1, U]]), m1, m2, MIN)        # Y2

        # ---- C: merge q-pairs -> Z[r(4):4U][j(4):U][u]
        Zb = wpool.tile([128, 1024], F16)
        Zs = wpool.tile([128, 1024], F16)
        Z = wpool.tile([128, 1024], F16)
        tt(_v(Zb, 0, [[4 * U, 4], [U, 4], [1, U]]),
           _v(YY, 0, [[8 * U, 4], [2 * U, 4], [1, U]]),
           _v(YY, 25 * U, [[-8 * U, 4], [2 * U, 4], [1, U]]), MAX)
        tt(_v(Zs, 0, [[4 * U, 2], [U, 4], [1, U]]),
           _v(Zb, 0, [[4 * U, 2], [U, 4], [1, U]]),
           _v(Zb, 8 * U, [[4 * U, 2], [U, 4], [1, U]]), MAX)
        tt(_v(Zs, 8 * U, [[4 * U, 2], [U, 4], [1, U]]),
           _v(Zb, 0, [[4 * U, 2], [U, 4], [1, U]]),
           _v(Zb, 8 * U, [[4 * U, 2], [U, 4], [1, U]]), MIN)
        tt(_v(Z, 0, [[8 * U, 2], [U, 4], [1, U]]),
           _v(Zs, 0, [[8 * U, 2], [U, 4], [1, U]]),
           _v(Zs, 4 * U, [[8 * U, 2], [U, 4], [1, U]]), MAX)
        tt(_v(Z, 4 * U, [[8 * U, 2], [U, 4], [1, U]]),
           _v(Zs, 0, [[8 * U, 2], [U, 4], [1, U]]),
           _v(Zs, 4 * U, [[8 * U, 2], [U, 4], [1, U]]), MIN)

        # ---- D: merge j-pairs -> Vt[r(4):2U][m(2):U][u]
        Vb = wpool.tile([128, 512], F16)
        Vs = wpool.tile([128, 512], F16)
        Vt = wpool.tile([128, 512], F16)
        tt(_v(Vb, 0, [[2 * U, 4], [U, 2], [1, U]]),
           _v(Z, 0, [[4 * U, 4], [2 * U, 2], [1, U]]),
           _v(Z, 13 * U, [[-4 * U, 4], [2 * U, 2], [1, U]]), MAX)
        tt(_v(Vs, 0, [[2 * U, 2], [U, 2], [1, U]]),
           _v(Vb, 0, [[2 * U, 2], [U, 2], [1, U]]),
           _v(Vb, 4 * U, [[2 * U, 2], [U, 2], [1, U]]), MAX)
        tt(_v(Vs, 4 * U, [[2 * U, 2], [U, 2], [1, U]]),
           _v(Vb, 0, [[2 * U, 2], [U, 2], [1, U]]),
           _v(Vb, 4 * U, [[2 * U, 2], [U, 2], [1, U]]), MIN)
        tt(_v(Vt, 0, [[4 * U, 2], [U, 2], [1, U]]),
           _v(Vs, 0, [[4 * U, 2], [U, 2], [1, U]]),
           _v(Vs, 2 * U, [[4 * U, 2], [U, 2], [1, U]]), MAX)
        tt(_v(Vt, 2 * U, [[4 * U, 2], [U, 2], [1, U]]),
           _v(Vs, 0, [[4 * U, 2], [U, 2], [1, U]]),
           _v(Vs, 2 * U, [[4 * U, 2], [U, 2], [1, U]]), MIN)

        # ---- E: final merge -> mm, scattered pb-major:
        # mm[pb*64 + r*16 + cb*4 + g]   (u = pb*16 + cb*4 + g)
        Mb = wpool.tile([128, 256], F16)
        Ms = wpool.tile([128, 256], F16)
        mm = wpool.tile([128, 256], F16)
        tt(_v(Mb, 0, [[U, 4], [1, U]]),
           _v(Vt, 0, [[2 * U, 4], [1, U]]),
           _v(Vt, 7 * U, [[-2 * U, 4], [1, U]]), MAX)
        tt(_v(Ms, 0, [[U, 2], [1, U]]),
           _v(Mb, 0, [[U, 2], [1, U]]),
           _v(Mb, 2 * U, [[U, 2], [1, U]]), MAX)
        tt(_v(Ms, 2 * U, [[U, 2], [1, U]]),
           _v(Mb, 0, [[U, 2], [1, U]]),
           _v(Mb, 2 * U, [[U, 2], [1, U]]), MIN)
        tt(_v(mm, 0, [[32, 2], [64, npb], [4, 4], [1, 4]]),
           _v(Ms, 0, [[2 * U, 2], [16, npb], [4, 4], [1, 4]]),
           _v(Ms, U, [[2 * U, 2], [16, npb], [4, 4], [1, 4]]), MAX)
        tt(_v(mm, 16, [[32, 2], [64, npb], [4, 4], [1, 4]]),
           _v(Ms, 0, [[2 * U, 2], [16, npb], [4, 4], [1, 4]]),
           _v(Ms, U, [[2 * U, 2], [16, npb], [4, 4], [1, 4]]), MIN)
        state[("mm", i)] = mm

    def convb(i, split_out=False):
        img, px0, npx = items[i]
        npb = npx // 128
        mm = state.pop(("mm", i))
        xt = state.pop(("xt", i))
        state.pop(("xv", i))

        # transpose top-4 planes back: mT[row = r*16+cb*4+g, pb*128+p]
        mT = mpool.tile([64, 512], F16)
        for pb in range(npb):
            psT = pst_pool.tile([64, 128], F16)
            nc.tensor.transpose(
                psT[:], _v(mm, pb * 64, [[1, 64]]), identh[:])
            nc.scalar.copy(_v(mT, pb * 128, [[1, 128]]), psT[:])

        # conv (fp16) + residual (f32r identity accumulate), evac, store
        osb = opool.tile([128, PB * 512], F32)
        base_o = out_d[:].offset + img * C * HW + px0
        for cb in range(4):
            psc = psc_pool.tile([128, 512], F32)
            nc.tensor.matmul(_v(psc, 0, [[1, npx]]),
                             _v(wbd2, cb * 128, [[1, 128]]),
                             _v(mT, 0, [[1, npx]]),
                             start=True, stop=False)
            nc.tensor.matmul(_v(psc, 0, [[1, npx]]), identr[:],
                             _v(xt, cb * npx, [[1, npx]]),
                             start=False, stop=True)
            nc.scalar.copy(_v(osb, cb * npx, [[1, npx]]), _v(psc, 0, [[1, npx]]))
            if split_out:
                dram_o = bass.AP(tensor=out_d.tensor,
                                 offset=base_o + cb * 128 * HW,
                                 ap=[[HW, 128], [1, npx]])
                nc.sync.dma_start(dram_o, _v(osb, cb * npx, [[1, npx]]))
        if not split_out:
            dram_o = bass.AP(tensor=out_d.tensor, offset=base_o,
                             ap=[[HW, 128], [128 * HW, 4], [1, npx]])
            nc.sync.dma_start(dram_o, _v(osb, 0, [[npx, 4], [1, npx]]))

    # warmups: first Act op triggers the activation-table load (1.3us) and a
    # cold PE runs matmuls 3.7x slower -- burn both costs during the first DMA
    warm = consts.tile([128, 128], F16)
    nc.scalar.copy(_v(warm, 0, [[1, 16]]), _v(identr, 0, [[1, 16]]))

    # software pipeline: tin runs two items ahead of sort so the Act
    # evacuation lands a full slot before the sort needs it; loads run three
    # ahead of sort (the DMA must complete before tin of that item).
    load(0, chunked=True)
    nc.sync.dma_start(identh[:], ih_d[:])
    nc.sync.dma_start(wbd2[:], w_d[:])
    load(1)
    tin(0)
    load(2)
    load(3)
    tin(1)
    for i in range(NIT):
        sortb(i)
        if i + 4 < NIT:
            load(i + 4)
        if i + 2 < NIT:
            tin(i + 2)
        convb(i, split_out=(i >= NIT - 2))


_NC_CACHE = None


def _get_nc():
    global _NC_CACHE
    if _NC_CACHE is None:
        _NC_CACHE = _build_nc()
    return _NC_CACHE


def _host_wbd2(w):
    # wbd2[r*16 + cb*4 + g', cb*128 + g'*32 + o] = w[cb*4+g', o, r]
    wbd2 = np.zeros((64, C), dtype=np.float32)
    for cb in range(4):
        for gp in range(4):
            g = cb * 4 + gp
            for r in range(4):
                row = r * 16 + cb * 4 + gp
                wbd2[row, cb * 128 + gp * 32: cb * 128 + (gp + 1) * 32] = w[g, :, r]
    return wbd2.astype(np.float16)


def run(x, w, trace=False):
    nc = _get_nc()
    xr = np.ascontiguousarray(x.astype(np.float32).reshape(N, C, HW))
    wbd2 = _host_wbd2(np.asarray(w, dtype=np.float32))
    identr = np.eye(128, dtype=np.float32)
    identh = np.eye(128, dtype=np.float16)
    in_maps = [
        {"x": xr[c * IMGS:(c + 1) * IMGS], "wbd2": wbd2,
         "identr": identr, "identh": identh}
        for c in range(NCORES)
    ]
    res = run_bass_kernel_spmd(nc, in_maps, core_ids=list(range(NCORES)), trace=trace)
    out = np.stack([r["out"] for r in res.results])  # [8, IMGS, C, HW]
    out = out.reshape(N, C, H, W)
    return out, res


def kernel(x, w):
    out, _ = run(x, w, trace=False)
    return out.astype(np.float32)


# revision 3
# speedup vs baseline: 1.0431x; 1.0431x over previous
"""GroupTopk Trainium2 kernel, v2.

x: [16, 512, 64, 64] f32. Per pixel, per group of 32 channels: top-4 values
(descending), grouped 1x1 conv [4 -> 32] with per-group weight w[g, o, k],
residual add. out = x + enhanced.

Strategy (8 cores, data-parallel over N, 2 images/core), per 512-pixel batch:
 - DMA x channel-major (contiguous 2KB lines) into an f32r-typed tile.
 - PE transposes (f32r, 1.5 cyc/row) into PSUM pixel-major.
 - Act evacuates PSUM -> SBUF as fp16 in "val-major" layout: plane v (channel
   within group) outer, unit u = (pixel-block, group) inner, so every sort op
   has stride-1 innermost access -> DVE 2x packed mode.
 - Bitonic top-4 merge network on DVE in fp16 (23 ops, all 2x-eligible).
 - PE transposes top-4 planes back (fp16, 1 cyc/row), fp16 conv matmul with
   block-diagonal weight + f32r identity residual-accumulate into PSUM.
 - Act evacuates PSUM -> SBUF f32; DMA out channel-major.
Queues: DVE = sort only; Act = evacuations; PE = transpose/matmul; SP = DMA.
"""

import numpy as np
from contextlib import ExitStack

import concourse.bacc as bacc
import concourse.bass as bass
import concourse.mybir as mybir
import concourse.tile as tile
from concourse.bass_utils import run_bass_kernel_spmd

F32 = mybir.dt.float32
F32R = mybir.dt.float32r
F16 = mybir.dt.float16

N, C, H, W = 16, 512, 64, 64
HW = H * W            # 4096
G, GS, K = 16, 32, 4  # groups, group size, topk
NCORES = 8
IMGS = N // NCORES    # images per core
PB = 4                # 128-pixel blocks per batch
BPX = PB * 128        # 512 pixels per batch
NBATCH = IMGS * HW // BPX

MAX = mybir.AluOpType.max
MIN = mybir.AluOpType.min


def _v(t, off, dims):
    """Strided view of a tile: keep partition dim, set free dims."""
    b = t[:]
    return bass.AP(
        tensor=b.tensor,
        offset=b.offset + off,
        ap=[list(b.ap[0])] + [list(d) for d in dims],
    )


def _build_nc():
    nc = bacc.Bacc("TRN2", target_bir_lowering=False, debug=False)
    x_d = nc.dram_tensor("x", [IMGS, C, HW], F32R, kind="ExternalInput").ap()
    w_d = nc.dram_tensor("wbd2", [64, C], F16, kind="ExternalInput").ap()
    ir_d = nc.dram_tensor("identr", [128, 128], F32R, kind="ExternalInput").ap()
    ih_d = nc.dram_tensor("identh", [128, 128], F16, kind="ExternalInput").ap()
    out_d = nc.dram_tensor("out", [IMGS, C, HW], F32, kind="ExternalOutput").ap()

    with tile.TileContext(nc) as tc:
        with ExitStack() as ctx:
            _emit(ctx, tc, nc, out_d, x_d, w_d, ir_d, ih_d)
    nc.compile()
    return nc


def _emit(ctx, tc, nc, out_d, x_d, w_d, ir_d, ih_d):
    consts = ctx.enter_context(tc.tile_pool(name="consts", bufs=1))
    xpool = ctx.enter_context(tc.tile_pool(name="xt", bufs=6))
    vpool = ctx.enter_context(tc.tile_pool(name="xv", bufs=3))
    wpool = ctx.enter_context(tc.tile_pool(name="work", bufs=3))
    mpool = ctx.enter_context(tc.tile_pool(name="mt", bufs=2))
    opool = ctx.enter_context(tc.tile_pool(name="osb", bufs=3))
    psx_pool = ctx.enter_context(tc.tile_pool(name="psx", bufs=1, space="PSUM"))
    pst_pool = ctx.enter_context(tc.tile_pool(name="pst", bufs=1, space="PSUM"))
    psc_pool = ctx.enter_context(tc.tile_pool(name="psc", bufs=3, space="PSUM"))

    # identr is needed immediately (warmup + first transposes); the other
    # consts are loaded after the first x chunks so they don't hold up the
    # serial HWDGE at kernel start
    identr = consts.tile([128, 128], F32R)
    nc.sync.dma_start(identr[:], ir_d[:])
    wbd2 = consts.tile([64, C], F16)
    identh = consts.tile([128, 128], F16)

    state = {}

    # work items: (img, px0, npx); first/last are half-size to shorten
    # pipeline fill and drain
    items = []
    items += [(0, 0, 256), (0, 256, 256)]
    items += [(0, 512 + k * 512, 512) for k in range(7)]
    items += [(1, k * 512, 512) for k in range(7)]
    items += [(1, 3584, 256), (1, 3840, 256)]
    NIT = len(items)

    def load(i, chunked=False):
        img, px0, npx = items[i]
        # xt[cc, cb*npx + px] = x[img, cb*128+cc, px0+px]
        xt = xpool.tile([128, PB * 512], F32R)
        base = x_d[:].offset + img * C * HW + px0
        if chunked:
            for cb in range(4):
                dram = bass.AP(tensor=x_d.tensor, offset=base + cb * 128 * HW,
                               ap=[[HW, 128], [1, npx]])
                nc.sync.dma_start(_v(xt, cb * npx, [[1, npx]]), dram)
        else:
            dram = bass.AP(tensor=x_d.tensor, offset=base,
                           ap=[[HW, 128], [128 * HW, 4], [1, npx]])
            nc.sync.dma_start(_v(xt, 0, [[npx, 4], [1, npx]]), dram)
        state[("xt", i)] = xt

    def tin(i):
        img, px0, npx = items[i]
        npb = npx // 128
        U = npx // 8
        xt = state[("xt", i)]
        # PE transpose -> psX[px_part, pb*512 + cb*128 + cc] (pixel-major, f32r)
        psX = psx_pool.tile([128, PB * 512], F32R)
        if i == 0:
            # dummy transposes with no DMA dependency: ramp the PE p-state
            # during the first input DMA (a cold PE runs 3.7x slower)
            for _ in range(4):
                nc.tensor.transpose(_v(psX, 0, [[1, 128]]), identr[:], identr[:])
        for cb in range(4):
            for pb in range(npb):
                src = _v(xt, cb * npx + pb * 128, [[1, 128]])   # [128cc, 128px]
                dst = _v(psX, pb * 512 + cb * 128, [[1, 128]])  # [128px, 128cc]
                nc.tensor.transpose(dst, src, identr[:])
        # evac: fp16 val-major scatter.  Xv[p, v*U + u], u = pb*16+cb*4+g
        # (on DVE for the first item: DVE is idle during fill and the A-stage
        # then follows on the same queue with no cross-engine handoff)
        Xv = vpool.tile([128, 2048], F16)
        src = _v(psX, 0, [[512, npb], [128, 4], [32, 4], [1, 32]])  # pb, cb, g, v
        dst = _v(Xv, 0, [[16, npb], [4, 4], [1, 4], [U, 32]])
        if i == 0:
            nc.vector.tensor_scalar_add(dst, src, 0.0)
        else:
            nc.scalar.copy(dst, src)
        state[("xv", i)] = Xv

    def sortb(i):
        img, px0, npx = items[i]
        npb = npx // 128
        U = npx // 8  # units per partition row
        Xv = state[("xv", i)]

        def tt(out, a, b, op):
            nc.vector.tensor_tensor(out, a, b, op)

        # ---- A: pairs -> PHL planes [PH(16) | PL(16)]
        PHL = wpool.tile([128, 2048], F16)
        Xe = _v(Xv, 0, [[2 * U, 16], [1, U]])
        Xo = _v(Xv, U, [[2 * U, 16], [1, U]])
        tt(_v(PHL, 0, [[U, 16], [1, U]]), Xe, Xo, MAX)
        tt(_v(PHL, 16 * U, [[U, 16], [1, U]]), Xe, Xo, MIN)

        # ---- B: sorted-4 lists, fused: one max op writes Y0 (rank0) + m2,
        # one min op writes m1 + Y3 (rank3); YY layout:
        # rank0 @0, rank1 @8U, rank2 @16U, rank3 @24U, m1 @32U, m2 @40U
        YY = wpool.tile([128, 3072], F16)
        He_and_Le = _v(PHL, 0, [[16 * U, 2], [2 * U, 8], [1, U]])
        Ho_and_Lo = _v(PHL, U, [[16 * U, 2], [2 * U, 8], [1, U]])
        tt(_v(YY, 0, [[40 * U, 2], [U, 8], [1, U]]),
           He_and_Le, Ho_and_Lo, MAX)                            # Y0, m2
        tt(_v(YY, 32 * U, [[-8 * U, 2], [U, 8], [1, U]]),
           He_and_Le, Ho_and_Lo, MIN)                            # m1, Y3
        m1 = _v(YY, 32 * U, [[U, 8], [1, U]])
        m2 = _v(YY, 40 * U, [[U, 8], [1, U]])
        tt(_v(YY, 8 * U, [[U, 8], [1, U]]), m1, m2, MAX)         # Y1
        tt(_v(YY, 16 * U, [[U, 8], [1, U]]), m1, m2, MIN)        # Y2

        # ---- C: merge q-pairs -> Z[r(4):4U][j(4):U][u]
        Zb = wpool.tile([128, 1024], F16)
        Zs = wpool.tile([128, 1024], F16)
        Z = wpool.tile([128, 1024], F16)
        tt(_v(Zb, 0, [[4 * U, 4], [U, 4], [1, U]]),
           _v(YY, 0, [[8 * U, 4], [2 * U, 4], [1, U]]),
           _v(YY, 25 * U, [[-8 * U, 4], [2 * U, 4], [1, U]]), MAX)
        tt(_v(Zs, 0, [[4 * U, 2], [U, 4], [1, U]]),
           _v(Zb, 0, [[4 * U, 2], [U, 4], [1, U]]),
           _v(Zb, 8 * U, [[4 * U, 2], [U, 4], [1, U]]), MAX)
        tt(_v(Zs, 8 * U, [[4 * U, 2], [U, 4], [1, U]]),
           _v(Zb, 0, [[4 * U, 2], [U, 4], [1, U]]),
           _v(Zb, 8 * U, [[4 * U, 2], [U, 4], [1, U]]), MIN)
        tt(_v(Z, 0, [[8 * U, 2], [U, 4], [1, U]]),
           _v(Zs, 0, [[8 * U, 2], [U, 4], [1, U]]),
           _v(Zs, 4 * U, [[8 * U, 2], [U, 4], [1, U]]), MAX)
        tt(_v(Z, 4 * U, [[8 * U, 2], [U, 4], [1, U]]),
           _v(Zs, 0, [[8 * U, 2], [U, 4], [1, U]]),
           _v(Zs, 4 * U, [[8 * U, 2], [U, 4], [1, U]]), MIN)

        # ---- D: merge j-pairs -> Vt[r(4):2U][m(2):U][u]
        Vb = wpool.tile([128, 512], F16)
        Vs = wpool.tile([128, 512], F16)
        Vt = wpool.tile([128, 512], F16)
        tt(_v(Vb, 0, [[2 * U, 4], [U, 2], [1, U]]),
           _v(Z, 0, [[4 * U, 4], [2 * U, 2], [1, U]]),
           _v(Z, 13 * U, [[-4 * U, 4], [2 * U, 2], [1, U]]), MAX)
        tt(_v(Vs, 0, [[2 * U, 2], [U, 2], [1, U]]),
           _v(Vb, 0, [[2 * U, 2], [U, 2], [1, U]]),
           _v(Vb, 4 * U, [[2 * U, 2], [U, 2], [1, U]]), MAX)
        tt(_v(Vs, 4 * U, [[2 * U, 2], [U, 2], [1, U]]),
           _v(Vb, 0, [[2 * U, 2], [U, 2], [1, U]]),
           _v(Vb, 4 * U, [[2 * U, 2], [U, 2], [1, U]]), MIN)
        tt(_v(Vt, 0, [[4 * U, 2], [U, 2], [1, U]]),
           _v(Vs, 0, [[4 * U, 2], [U, 2], [1, U]]),
           _v(Vs, 2 * U, [[4 * U, 2], [U, 2], [1, U]]), MAX)
        tt(_v(Vt, 2 * U, [[4 * U, 2], [U, 2], [1, U]]),
           _v(Vs, 0, [[4 * U, 2], [U, 2], [1, U]]),
           _v(Vs, 2 * U, [[4 * U, 2], [U, 2], [1, U]]), MIN)

        # ---- E: final merge -> mm, scattered pb-major:
        # mm[pb*64 + r*16 + cb*4 + g]   (u = pb*16 + cb*4 + g)
        Mb = wpool.tile([128, 256], F16)
        Ms = wpool.tile([128, 256], F16)
        mm = wpool.tile([128, 256], F16)
        tt(_v(Mb, 0, [[U, 4], [1, U]]),
           _v(Vt, 0, [[2 * U, 4], [1, U]]),
           _v(Vt, 7 * U, [[-2 * U, 4], [1, U]]), MAX)
        tt(_v(Ms, 0, [[U, 2], [1, U]]),
           _v(Mb, 0, [[U, 2], [1, U]]),
           _v(Mb, 2 * U, [[U, 2], [1, U]]), MAX)
        tt(_v(Ms, 2 * U, [[U, 2], [1, U]]),
           _v(Mb, 0, [[U, 2], [1, U]]),
           _v(Mb, 2 * U, [[U, 2], [1, U]]), MIN)
        tt(_v(mm, 0, [[32, 2], [64, npb], [4, 4], [1, 4]]),
           _v(Ms, 0, [[2 * U, 2], [16, npb], [4, 4], [1, 4]]),
           _v(Ms, U, [[2 * U, 2], [16, npb], [4, 4], [1, 4]]), MAX)
        tt(_v(mm, 16, [[32, 2], [64, npb], [4, 4], [1, 4]]),
           _v(Ms, 0, [[2 * U, 2], [16, npb], [4, 4], [1, 4]]),
           _v(Ms, U, [[2 * U, 2], [16, npb], [4, 4], [1, 4]]), MIN)
        state[("mm", i)] = mm

    def convb(i, split_out=False, dve_evac=False):
        img, px0, npx = items[i]
        npb = npx // 128
        mm = state.pop(("mm", i))
        xt = state.pop(("xt", i))
        state.pop(("xv", i))

        nevac = [0]

        def evac(dst, src):
            # drain-zone items alternate PSUM evacuations between the (then
            # idle) DVE and Act so both engines chew the tail in parallel
            nevac[0] += 1
            if dve_evac and nevac[0] % 2:
                nc.vector.tensor_scalar_add(dst, src, 0.0)
            else:
                nc.scalar.copy(dst, src)

        # transpose top-4 planes back: mT[row = r*16+cb*4+g, pb*128+p]
        mT = mpool.tile([64, 512], F16)
        for pb in range(npb):
            psT = pst_pool.tile([64, 128], F16)
            nc.tensor.transpose(
                psT[:], _v(mm, pb * 64, [[1, 64]]), identh[:])
            evac(_v(mT, pb * 128, [[1, 128]]), psT[:])

        # conv (fp16) + residual (f32r identity accumulate), evac, store
        osb = opool.tile([128, PB * 512], F32)
        base_o = out_d[:].offset + img * C * HW + px0
        for cb in range(4):
            psc = psc_pool.tile([128, 512], F32)
            nc.tensor.matmul(_v(psc, 0, [[1, npx]]),
                             _v(wbd2, cb * 128, [[1, 128]]),
                             _v(mT, 0, [[1, npx]]),
                             start=True, stop=False)
            nc.tensor.matmul(_v(psc, 0, [[1, npx]]), identr[:],
                             _v(xt, cb * npx, [[1, npx]]),
                             start=False, stop=True)
            evac(_v(osb, cb * npx, [[1, npx]]), _v(psc, 0, [[1, npx]]))
            if split_out:
                dram_o = bass.AP(tensor=out_d.tensor,
                                 offset=base_o + cb * 128 * HW,
                                 ap=[[HW, 128], [1, npx]])
                nc.sync.dma_start(dram_o, _v(osb, cb * npx, [[1, npx]]))
        if not split_out:
            dram_o = bass.AP(tensor=out_d.tensor, offset=base_o,
                             ap=[[HW, 128], [128 * HW, 4], [1, npx]])
            nc.sync.dma_start(dram_o, _v(osb, 0, [[npx, 4], [1, npx]]))

    # warmups: first Act op triggers the activation-table load (1.3us) and a
    # cold PE runs matmuls 3.7x slower -- burn both costs during the first DMA
    warm = consts.tile([128, 128], F16)
    nc.scalar.copy(_v(warm, 0, [[1, 16]]), _v(identr, 0, [[1, 16]]))

    # software pipeline: tin runs two items ahead of sort so the Act
    # evacuation lands a full slot before the sort needs it; loads run three
    # ahead of sort (the DMA must complete before tin of that item).
    load(0, chunked=True)
    nc.sync.dma_start(identh[:], ih_d[:])
    nc.sync.dma_start(wbd2[:], w_d[:])
    load(1)
    tin(0)
    load(2)
    load(3)
    tin(1)
    # the last 3 items' conv work is emitted after every sort so their DVE
    # evacuations queue behind the final sorts, not in front of them
    DEFER = 3
    for i in range(NIT):
        sortb(i)
        if i + 4 < NIT:
            load(i + 4)
        if i + 2 < NIT:
            tin(i + 2)
        if i < NIT - DEFER:
            convb(i)
    for i in range(NIT - DEFER, NIT):
        convb(i, split_out=(i >= NIT - 2), dve_evac=True)


_NC_CACHE = None


def _get_nc():
    global _NC_CACHE
    if _NC_CACHE is None:
        _NC_CACHE = _build_nc()
    return _NC_CACHE


def _host_wbd2(w):
    # wbd2[r*16 + cb*4 + g', cb*128 + g'*32 + o] = w[cb*4+g', o, r]
    wbd2 = np.zeros((64, C), dtype=np.float32)
    for cb in range(4):
        for gp in range(4):
            g = cb * 4 + gp
            for r in range(4):
                row = r * 16 + cb * 4 + gp
                wbd2[row, cb * 128 + gp * 32: cb * 128 + (gp + 1) * 32] = w[g, :, r]
    return wbd2.astype(np.float16)


def run(x, w, trace=False):
    nc = _get_nc()
    xr = np.ascontiguousarray(x.astype(np.float32).reshape(N, C, HW))
    wbd2 = _host_wbd2(np.asarray(w, dtype=np.float32))
    identr = np.eye(128, dtype=np.float32)
    identh = np.eye(128, dtype=np.float16)
    in_maps = [
        {"x": xr[c * IMGS:(c + 1) * IMGS], "wbd2": wbd2,
         "identr": identr, "identh": identh}
        for c in range(NCORES)
    ]
    res = run_bass_kernel_spmd(nc, in_maps, core_ids=list(range(NCORES)), trace=trace)
    out = np.stack([r["out"] for r in res.results])  # [8, IMGS, C, HW]
    out = out.reshape(N, C, H, W)
    return out, res


def kernel(x, w):
    out, _ = run(x, w, trace=False)
    return out.astype(np.float32)
